# revision 20
# baseline (speedup 1.0000x reference)
"""LIF readout kernel for Trainium2 (8 NeuronCores, data-parallel over batch).

Reference computation (per element):
    cur[t,b,o] = (x[t] @ W)[b,o] + bias_o + psp          (psp = THRESH/(2T))
    v_t   = DECAY*m_{t-1} + cur_t
    s_t   = (v_t > THRESH)
    m_t   = v_t - s_t*THRESH
    out[b,o] = mean_t s_t

Device strategy per core (B_local = 16 batch rows):
  - GEMM out.T orientation: psum[o_p, (t,b)] = W[:,j*128:+128].T @ xT-block,
    K=2048 via 16 accumulating matmuls into fp32 psum.  x and W are fed as
    float16 (11-bit mantissa; x~N(0,1), W~N(0,1/sqrt(C)) fit comfortably):
    halves DMA traffic, runs 1 cycle/row at any free size, and enables fast
    weight loads.
  - W is laid out j-major and DMA'd in 8 chunks; the first x block goes
    first so matmuls start as soon as chunk j0 lands.
  - ScalarE copies each psum j-tile into an SBUF currents ring laid out
    [o_p, (t, j, b)] fp16, so one scan step reads a contiguous [128,128]
    slice.
  - VectorE runs the scan as 2 fp16 scalar_tensor_tensor ops per step
    (n = -(m - kappa), kappa = 10*psp, THETA = THRESH - kappa):
        sv     = (n_{t-1} * -DECAY) + cur[t]    # v_t
        cur[t] = (sv is_gt THETA) - sv          # n_t overwrites c_t in place
    while GpSimd accumulates N = sum_t n_t from the ring in parallel.
    Reading n from the ring means no per-step cross-engine write hazard,
    so VectorE runs semaphore-free.  Spike counts are recovered at the
    end from the exact identity
        sum_t s_t = (1-DECAY)*N + DECAY*n_{T-1} - DECAY*kappa + sum_t c_t
    sum_t c_t comes from 16 extra moving columns (host-precomputed
    sum_t x) appended to block XBLK's GEMM.  Membrane quantization at
    2^-11 flips ~0.5% extra near-threshold spikes (rel L2 ~6e-3, well
    inside tolerance).
  - Output: spike counts [o_p=128, (j,b)=128] fp32 DMA'd raw; host
    un-permutes and divides by T.
"""
import numpy as np
from contextlib import ExitStack

import concourse.bass as bass
import concourse.tile as tile
from concourse import bacc, mybir
from concourse.bass_utils import run_bass_kernel_spmd

T, B, C, O = 100, 128, 2048, 1000
NCORES = 8
BL = B // NCORES            # 16 batch rows per core
OP = 1024                   # O padded to 8 o-chunks of 128
NJ = OP // 128              # 8
NK = C // 128               # 16
DECAY = 0.9
THRESH = 1.0
PSP = THRESH / (2 * T)      # 0.005
KAPPA = PSP / (1.0 - DECAY)     # 0.05
THETA = THRESH - KAPPA          # 0.95

# t-steps per pipeline block (m-col width = 16*t <= 512, the psum bank
# limit).  The GEMM is purely matmul-paced at ~0.85us/step vs the scan's
# ~0.75us/step, so the scan always trails by up to a block; block sizes
# taper so each block's bound (currents-ready + remaining scan) is even.
TBLOCKS = [24, 28, 26, 22]
XBLK = 0
# W DMA trigger groups (each descriptor costs ~0.6-1.2us of issue time on
# the sync queue): j0 alone unblocks the first matmul group ASAP, the rest
# ride in pairs.
WGROUPS = [[0], [1, 2], [3, 4], [5, 6], [7]]
assert sum(TBLOCKS) == T
assert all(16 * tb <= 512 for tb in TBLOCKS)
assert 16 * TBLOCKS[XBLK] + 16 <= 512

F32 = mybir.dt.float32
F16 = mybir.dt.float16

_cache: dict = {}


def _build(use_bias: bool):
    nc = bacc.Bacc("TRN2", target_bir_lowering=False, debug=False)

    total_cols = NK * (BL * T + 16)   # + xsum columns riding with block XBLK
    x_d = nc.dram_tensor("xp", [128, total_cols], F16, kind="ExternalInput")
    w_d = nc.dram_tensor("wp", [128, NJ * NK * 128], F16, kind="ExternalInput")
    if use_bias:
        b_d = nc.dram_tensor("bp", [1, OP], F16, kind="ExternalInput")
    o_d = nc.dram_tensor("acc_raw", [128, 128], F32, kind="ExternalOutput")

    def sx_of(bi):
        return BL * TBLOCKS[bi] + (16 if bi == XBLK else 0)

    with tile.TileContext(nc) as tc, ExitStack() as ctx:
        wpool = ctx.enter_context(tc.tile_pool(name="wpool", bufs=1))
        xpool = ctx.enter_context(tc.tile_pool(name="xpool", bufs=3))
        cpool = ctx.enter_context(tc.tile_pool(name="cpool", bufs=3))
        spool = ctx.enter_context(tc.tile_pool(name="spool", bufs=1))
        ppool = ctx.enter_context(tc.tile_pool(name="ppool", bufs=1, space="PSUM"))

        # DMA order: first x block, then the W chunks (group [j0] unblocks
        # the first matmul group ASAP), then the remaining x blocks.
        xts = []
        xt0 = xpool.tile([128, NK * sx_of(0)], F16, tag="xt", name="xt0")
        nc.sync.dma_start(xt0[:], x_d[:, 0:NK * sx_of(0)])
        xts.append(xt0)

        wts = [None] * NJ          # (tile, col offset of j's chunk)
        for gi, grp in enumerate(WGROUPS):
            j0g = grp[0]
            wg = wpool.tile([128, len(grp) * NK * 128], F16, name=f"wg{gi}")
            nc.sync.dma_start(
                wg[:], w_d[:, j0g * NK * 128:(j0g + len(grp)) * NK * 128])
            for oi, j in enumerate(grp):
                wts[j] = (wg, oi * NK * 128)
        if use_bias:
            bt = wpool.tile([1, OP], F16, name="bt")
            nc.sync.dma_start(bt[:], b_d[:])
            ones = wpool.tile([1, 544], F16, name="ones")
            nc.vector.memset(ones[:], 1.0)
            # the xsum columns need T*b, not b
            oxs = BL * TBLOCKS[XBLK]
            nc.vector.memset(ones[:, oxs:oxs + 16], float(T))

        coff = NK * sx_of(0)
        for bi in range(1, len(TBLOCKS)):
            Sx = sx_of(bi)
            xt = xpool.tile([128, NK * Sx], F16, tag="xt", name=f"xt{bi}")
            nc.sync.dma_start(xt[:], x_d[:, coff:coff + NK * Sx])
            xts.append(xt)
            coff += NK * Sx

        sv = spool.tile([128, 128], F16, name="sv")
        ninit = spool.tile([128, 128], F16, name="ninit")
        nsum = spool.tile([128, 128], F32, name="nsum")
        csum = spool.tile([128, 128], F32, name="csum")
        nc.vector.memset(ninit[:], KAPPA)   # n_{-1}
        nc.gpsimd.memset(nsum[:], 0.0)

        prev_n = ninit[:]
        for bi, tb in enumerate(TBLOCKS):
            S = BL * tb
            Sx = sx_of(bi)
            xt = xts[bi]
            cur = cpool.tile([128, tb * 128], F16, tag="cur", name=f"cur{bi}")
            cur3 = cur[:].rearrange("p (t v) -> p t v", v=128)
            for j in range(NJ):
                ps = ppool.tile([128, Sx], F32, tag=f"ps{j}", name=f"ps{bi}_{j}")
                wg, wo = wts[j]
                for k in range(NK):
                    nc.tensor.matmul(
                        ps[:],
                        wg[:, wo + k * 128:wo + (k + 1) * 128],
                        xt[:, k * Sx:(k + 1) * Sx],
                        start=(k == 0),
                        stop=(k == NK - 1 and not use_bias),
                    )
                if use_bias:
                    nc.tensor.matmul(
                        ps[:],
                        bt[:, j * 128:(j + 1) * 128],
                        ones[:, :Sx],
                        start=False,
                        stop=True,
                    )
                # psum [o_p,(t,b)] fp32 -> currents ring [o_p,(t,j,b)] fp16
                nc.scalar.copy(
                    cur3[:, :, j * BL:(j + 1) * BL],
                    ps[:, :S].rearrange("p (t b) -> p t b", b=BL),
                )
                if bi == XBLK:
                    nc.scalar.copy(csum[:, j * 16:(j + 1) * 16], ps[:, S:Sx])

            # scan: v_t -> sv, n_t -> a parallel ring (clean writes avoid
            # the read-modify-write penalty of overwriting cur[t]).
            # GpSimd reads n_t from the ring, so the only V<->G hazard is
            # the coarse per-block ring-buffer reuse edge.
            nring = cpool.tile([128, tb * 128], F16, tag="nring", name=f"nr{bi}")
            for tl in range(tb):
                c_t = cur[:, tl * 128:(tl + 1) * 128]
                n_t = nring[:, tl * 128:(tl + 1) * 128]
                nc.vector.scalar_tensor_tensor(
                    out=sv[:], in0=prev_n, scalar=-DECAY, in1=c_t,
                    op0=mybir.AluOpType.mult, op1=mybir.AluOpType.add)
                nc.vector.scalar_tensor_tensor(
                    out=n_t, in0=sv[:], scalar=THETA, in1=sv[:],
                    op0=mybir.AluOpType.is_gt, op1=mybir.AluOpType.subtract)
                nc.gpsimd.tensor_tensor(
                    out=nsum[:], in0=nsum[:], in1=n_t,
                    op=mybir.AluOpType.add)
                prev_n = n_t

        # combine: out = (1-d)*N + d*n_{T-1} - d*kappa + csum
        nc.vector.scalar_tensor_tensor(
            out=nsum[:], in0=nsum[:], scalar=1.0 - DECAY, in1=csum[:],
            op0=mybir.AluOpType.mult, op1=mybir.AluOpType.add)
        nc.vector.scalar_tensor_tensor(
            out=nsum[:], in0=prev_n, scalar=DECAY, in1=nsum[:],
            op0=mybir.AluOpType.mult, op1=mybir.AluOpType.add)
        nc.vector.tensor_scalar(
            out=nsum[:], in0=nsum[:], scalar1=-DECAY * KAPPA, scalar2=None,
            op0=mybir.AluOpType.add)

        nc.sync.dma_start(o_d[:], nsum[:])

    nc.finalize()
    return nc


def _prep_x(x_core: np.ndarray) -> np.ndarray:
    """x_core [T, BL, C] -> fp16 [128, cols] block-major (k, m) layout, with
    sum_t x appended as 16 extra m-columns per k-slice of block XBLK."""
    xm = np.ascontiguousarray(x_core.reshape(T * BL, C).T)   # [C, M] = [k*128+p, m]
    xk = xm.reshape(NK, 128, T * BL)                         # [k, p, m]
    xs = x_core.sum(axis=0, dtype=np.float64).T.astype(np.float32)  # [C, BL]
    xsk = xs.reshape(NK, 128, BL)                            # [k, p, b]
    segs = []
    m0 = 0
    for bi, tb in enumerate(TBLOCKS):
        S = BL * tb
        seg = xk[:, :, m0:m0 + S]                            # [k, p, S]
        if bi == XBLK:
            seg = np.concatenate([seg, xsk], axis=2)         # [k, p, S+16]
        segs.append(np.ascontiguousarray(seg.transpose(1, 0, 2)).reshape(128, -1))
        m0 += S
    return np.concatenate(segs, axis=1).astype(np.float16)


def kernel(x: np.ndarray, W: np.ndarray, b: np.ndarray) -> np.ndarray:
    x = np.asarray(x, dtype=np.float32)
    W = np.asarray(W, dtype=np.float32)
    b = np.asarray(b, dtype=np.float32)
    use_bias = bool(np.any(b != 0.0))

    key = use_bias
    if key not in _cache:
        _cache[key] = _build(use_bias)
    nc = _cache[key]

    Wp = np.zeros((C, OP), np.float32)
    Wp[:, :O] = W
    # j-major chunks: [128, (j, k, 128)]
    wprep = np.ascontiguousarray(
        Wp.reshape(NK, 128, NJ, 128).transpose(1, 2, 0, 3)
          .reshape(128, NJ * NK * 128)).astype(np.float16)

    in_maps = []
    for c in range(NCORES):
        m = {"xp": _prep_x(x[:, c * BL:(c + 1) * BL, :]), "wp": wprep}
        if use_bias:
            bp = np.zeros((1, OP), np.float16)
            bp[0, :O] = b.astype(np.float16)
            m["bp"] = bp
        in_maps.append(m)

    res = run_bass_kernel_spmd(nc, in_maps, list(range(NCORES)))

    outs = []
    for c in range(NCORES):
        raw = res.results[c]["acc_raw"]                      # [o_p, (j, b)]
        rate = raw.reshape(128, NJ, BL).transpose(2, 1, 0).reshape(BL, OP)
        outs.append(rate[:, :O] / np.float32(T))
    return np.concatenate(outs, axis=0).astype(np.float32)
